# revision 1
# baseline (speedup 1.0000x reference)
"""Trainium2 Bass kernel for nn_CrossAttention (B=2, TGT=1024, SRC=2048,
H=1024, 16 heads x 64).

Sharding: 8 cores = 2 (batch) x 4 (head groups of 4 heads). Each core
computes q/k/v projections for its 4 heads (column-sliced weights), the
attention for those heads, and a partial out-projection (row-sliced Wo).
The host sums the 4 partial out-projections per batch (the "all-reduce")
and adds bo.

On-device layout is fully transposed ("contraction dim on partitions")
so no on-device transposes are needed:
  qT [d, t], kT [d, s]  from projection matmuls (host supplies x^T, W^T)
  logitsT [s, t] per-head matmul (K = head_dim = 64); the attn bias is
  accumulated into the logits PSUM by a second matmul with an identity
  stationary operand (PE does the add; DVE stays off the critical path)
  pT = exp(logitsT + biasT) via ScalarE straight out of PSUM (mask
  pre-folded into bias on host as -3e4; softmax max-subtraction skipped:
  logits are O(10), safe in fp32)
  attnT [d, t] = PV matmul with V augmented by 64 ones-columns, so
  rows 64..127 of the PV psum hold the softmax denominator replicated;
  a DVE reciprocal+mul normalizes with no cross-partition broadcast.

Matmul operands are float32r (TF32-like, full PE rate at free-dim>=256);
accumulation stays fp32 in PSUM. The two heads sharing a 128-partition
tile run their K=64 QK matmuls in disjoint PE row groups (concurrent).
"""

import numpy as np
from contextlib import ExitStack

import concourse.bass as bass
import concourse.tile as tile
from concourse import bacc, mybir
from concourse.bass_utils import run_bass_kernel_spmd

P = 128
H_DIM = 1024
N_HEADS = 16
HEAD_DIM = 64
B = 2
TGT = 1024
SRC = 2048
N_CORES = 8
HPC = 4  # heads per core
DPC = HPC * HEAD_DIM  # 256 projected dims per core
F32 = mybir.dt.float32
F32R = mybir.dt.float32r
BF16 = mybir.dt.bfloat16

NEG_MASK = -30000.0
TQ = 512  # t-chunk for attention units
S_TILES = SRC // P  # 16
KT = H_DIM // P  # 8 contraction tiles for projections
DT = DPC // P  # 2 d-tiles per core

_prog_cache: dict = {}


def _emit(tc: tile.TileContext, outs, ins):
    nc = tc.nc

    def rview(dram_ap):
        # fp32r is a bit-identical reinterpretation of fp32 DRAM data
        return dram_ap.bitcast(F32R)

    xqT, xkT, xvT, biasT, wqT, wkT, wvT, woT, bqv, bkv, bvv, identity = ins
    (outT,) = outs

    with ExitStack() as ctx:
        const = ctx.enter_context(tc.tile_pool(name="const", bufs=1))
        xpool = ctx.enter_context(tc.tile_pool(name="xin", bufs=4))
        xvpool = ctx.enter_context(tc.tile_pool(name="xvin", bufs=6))
        biasp = ctx.enter_context(tc.tile_pool(name="biasin", bufs=4))
        ppool = ctx.enter_context(tc.tile_pool(name="pT", bufs=2))
        rcpool = ctx.enter_context(tc.tile_pool(name="rcp", bufs=2))
        outp = ctx.enter_context(tc.tile_pool(name="outsb", bufs=3))
        psA = ctx.enter_context(tc.tile_pool(name="psA", bufs=2, space="PSUM"))
        psL = ctx.enter_context(tc.tile_pool(name="psL", bufs=1, space="PSUM"))
        psV = ctx.enter_context(tc.tile_pool(name="psV", bufs=1, space="PSUM"))

        # ---- persistent SBUF tensors (matmul operands in fp32r) ----
        wq_sb = const.tile([P, KT, DPC], F32R)  # [e_part, e_tile, d]
        wk_sb = const.tile([P, KT, DPC], F32R)
        wv_sb = const.tile([P, KT, DPC], F32R)
        wo_sb = const.tile([P, DT, H_DIM], F32R)  # [hd_part, hd_tile, e_out]
        ident = const.tile([P, P], F32R)
        bq_sb = const.tile([P, DT], F32)
        bk_sb = const.tile([P, DT], F32)
        bv_bc = const.tile([P, DPC], F32)  # bv broadcast down partitions
        q_sb = const.tile([P, DT, TGT], F32R)  # qT
        k_sb = const.tile([P, DT, SRC], F32R)  # kT
        # v plus 64 ones-columns, per (s_tile, head): [.., 0:64]=v, [.., 64:128]=1
        v_sb = const.tile([P, S_TILES, HPC, P], BF16)
        attn_sb = const.tile([P, DT, TGT], F32R)  # attnT, normalized

        nc.sync.dma_start(wq_sb[:], rview(wqT.rearrange("(k p) d -> p k d", p=P)))
        nc.sync.dma_start(wk_sb[:], rview(wkT.rearrange("(k p) d -> p k d", p=P)))
        nc.sync.dma_start(wv_sb[:], rview(wvT.rearrange("(k p) d -> p k d", p=P)))
        nc.sync.dma_start(wo_sb[:], rview(woT.rearrange("(m p) e -> p m e", p=P)))
        nc.sync.dma_start(ident[:], rview(identity))
        nc.sync.dma_start(bq_sb[:], bqv.rearrange("(m p) -> p m", p=P))
        nc.sync.dma_start(bk_sb[:], bkv.rearrange("(m p) -> p m", p=P))
        nc.sync.dma_start(bv_bc[:], bvv[None, :].to_broadcast((P, DPC)))
        ones_region = v_sb[:, :, :, HEAD_DIM:P]
        nc.vector.tensor_copy(
            ones_region, nc.const_aps.tensor(1.0, ones_region.shape, F32))

        # ---- q/k projections: psum[d_tile] += wT_tile.T @ xT_tile ----
        def proj_qk(x_dram, w_sb, b_sb, dst_sb, LEN):
            for n in range(LEN // 512):
                pss = [psA.tile([P, 512], F32, name=f"pjq{m}", tag="mm") for m in range(DT)]
                for k in range(KT):
                    xt = xpool.tile([P, 512], F32R, name="xt")
                    nc.sync.dma_start(
                        xt[:], rview(x_dram[k * P:(k + 1) * P, n * 512:(n + 1) * 512]))
                    for m in range(DT):
                        nc.tensor.matmul(
                            pss[m][:],
                            lhsT=w_sb[:, k, m * P:(m + 1) * P],
                            rhs=xt[:],
                            start=(k == 0),
                            stop=(k == KT - 1),
                        )
                for m in range(DT):
                    nc.scalar.activation(
                        dst_sb[:, m, n * 512:(n + 1) * 512],
                        pss[m][:],
                        mybir.ActivationFunctionType.Identity,
                        bias=b_sb[:, m:m + 1],
                    )

        with nc.named_scope("proj_q"):
            proj_qk(xqT, wq_sb, bq_sb, q_sb, TGT)
        with nc.named_scope("proj_k"):
            proj_qk(xkT, wk_sb, bk_sb, k_sb, SRC)

        # ---- v projection: psum[s_tile] += xvT_tile.T @ wv ----
        vs = nc.enter_named_scope("proj_v", False)
        for m in range(S_TILES):
            ps = psA.tile([P, 512], F32, name="pjv", tag="mm")[:, :DPC]
            for k in range(KT):
                xvt = xvpool.tile([P, P], F32R, name="xvt")
                nc.sync.dma_start(
                    xvt[:], rview(xvT[k * P:(k + 1) * P, m * P:(m + 1) * P]))
                nc.tensor.matmul(
                    ps,
                    lhsT=xvt[:],
                    rhs=wv_sb[:, k, :],
                    start=(k == 0),
                    stop=(k == KT - 1),
                )
            nc.vector.tensor_add(
                v_sb[:, m, :, 0:HEAD_DIM],
                ps.rearrange("p (h d) -> p h d", d=HEAD_DIM),
                bv_bc.rearrange("p (h d) -> p h d", d=HEAD_DIM),
            )
        nc.leave_named_scope("proj_v", vs[0], False)

        # ---- attention units: (head-pair, t-chunk), software-pipelined ----
        # Per unit: 8 groups of 2 s-tiles. Each group: 4 QK matmuls (the two
        # heads of the pair run in disjoint PE row groups), 4 identity-matmuls
        # accumulating the bias into the logits PSUM, 4 PV matmuls of the
        # PREVIOUS unit (they fill the PE while ScalarE drains this group's
        # exp), then 2 batched exps. PV probabilities are bf16 (rounding noise
        # averages out over the 2048-key sum).
        units = [(pair, tc_i) for pair in range(HPC // 2)
                 for tc_i in range(TGT // TQ)]
        prev = None  # (p_tiles, pair, t_sl)

        def emit_pv_chunk(state, chunk):
            p_tiles_p, pair_p, t_sl_p = state["unit"]
            if state["pvs"] is None:
                state["pvs"] = [
                    psV.tile([P, TQ], F32, name=f"pv{j}", tag=f"pv{j}")
                    for j in range(2)
                ]
            for (j, m) in chunk:
                h = 2 * pair_p + j
                nc.tensor.matmul(
                    state["pvs"][j][:],
                    lhsT=v_sb[:, m, h, :],
                    rhs=p_tiles_p[j][:, m, :],
                    start=(m == 0),
                    stop=(m == S_TILES - 1),
                )

        def finish_pv(state):
            p_tiles_p, pair_p, t_sl_p = state["unit"]
            for j in range(2):
                p0 = j * HEAD_DIM
                rc = rcpool.tile([P, TQ], F32, name="rc")
                nc.vector.reciprocal(rc[HEAD_DIM:P, :], state["pvs"][j][HEAD_DIM:P, :])
                nc.vector.tensor_mul(
                    attn_sb[p0:p0 + HEAD_DIM, pair_p, t_sl_p],
                    state["pvs"][j][0:HEAD_DIM, :],
                    rc[HEAD_DIM:P, :]
                )

        for pair, tc_i in units:
            t_sl = slice(tc_i * TQ, (tc_i + 1) * TQ)
            sc = nc.enter_named_scope(f"attn_p{pair}t{tc_i}", False)
            p_tiles = [
                ppool.tile([P, S_TILES, TQ], BF16, name=f"p_t{j}", tag=f"pT{j}")
                for j in range(2)
            ]
            pv_sched = [(j, m) for m in range(S_TILES) for j in range(2)]
            for g in range(S_TILES // 2):
                ms = 2 * g
                pls = [
                    psL.tile([P, 2, TQ], F32, name=f"lg{j}", tag=f"lg{j}")
                    for j in range(2)
                ]
                bts = [
                    biasp.tile([P, 2, TQ], F32R, name=f"bt{j}", tag="bt")
                    for j in range(2)
                ]
                for j in range(2):
                    h = 2 * pair + j
                    nc.sync.dma_start(
                        bts[j][:],
                        rview(biasT[h, ms * P:(ms + 2) * P, t_sl]).rearrange(
                            "(mm p) t -> p mm t", p=P))
                for mi in range(2):
                    for j in range(2):
                        p0 = j * HEAD_DIM
                        nc.tensor.matmul(
                            pls[j][:, mi, :],
                            lhsT=k_sb[p0:p0 + HEAD_DIM, pair, (ms + mi) * P:(ms + mi + 1) * P],
                            rhs=q_sb[p0:p0 + HEAD_DIM, pair, t_sl],
                            start=True,
                            stop=False,
                        )
                for j in range(2):
                    for mi in range(2):
                        nc.tensor.matmul(
                            pls[j][:, mi, :],
                            lhsT=ident[:],
                            rhs=bts[j][:, mi, :],
                            start=False,
                            stop=True,
                        )
                if prev is not None:
                    emit_pv_chunk(prev, pv_sched[4 * g:4 * g + 4])
                for j in range(2):
                    nc.scalar.activation(
                        p_tiles[j][:, ms:ms + 2, :], pls[j][:],
                        mybir.ActivationFunctionType.Exp,
                    )
            if prev is not None:
                finish_pv(prev)
            prev = {"unit": (p_tiles, pair, t_sl), "pvs": None}
            nc.leave_named_scope(f"attn_p{pair}t{tc_i}", sc[0], False)

        # drain the last unit's PV
        pv_sched = [(j, m) for m in range(S_TILES) for j in range(2)]
        for c in range(0, len(pv_sched), 4):
            emit_pv_chunk(prev, pv_sched[c:c + 4])
        finish_pv(prev)

        # ---- out projection (partial; host sums across head groups) ----
        osc = nc.enter_named_scope("outproj", False)
        for mo in range(H_DIM // P):
            for n in range(TGT // 512):
                ps = psA.tile([P, 512], F32, name="po", tag="mm")
                for kt in range(DT):
                    nc.tensor.matmul(
                        ps[:],
                        lhsT=wo_sb[:, kt, mo * P:(mo + 1) * P],
                        rhs=attn_sb[:, kt, n * 512:(n + 1) * 512],
                        start=(kt == 0),
                        stop=(kt == DT - 1),
                    )
                ot = outp.tile([P, 512], F32, name="ot")
                nc.scalar.activation(ot[:], ps[:],
                                     mybir.ActivationFunctionType.Copy)
                nc.sync.dma_start(outT[mo * P:(mo + 1) * P, n * 512:(n + 1) * 512], ot[:])
        nc.leave_named_scope("outproj", osc[0], False)


def _build_program():
    key = ("prog", "f32r_pe_bias")
    if key in _prog_cache:
        return _prog_cache[key]
    nc = bacc.Bacc("TRN2", target_bir_lowering=False, debug=False, num_devices=N_CORES)
    ins = [
        nc.dram_tensor("xqT", [H_DIM, TGT], F32, kind="ExternalInput").ap(),
        nc.dram_tensor("xkT", [H_DIM, SRC], F32, kind="ExternalInput").ap(),
        nc.dram_tensor("xvT", [H_DIM, SRC], F32, kind="ExternalInput").ap(),
        nc.dram_tensor("biasT", [HPC, SRC, TGT], F32, kind="ExternalInput").ap(),
        nc.dram_tensor("wqT", [H_DIM, DPC], F32, kind="ExternalInput").ap(),
        nc.dram_tensor("wkT", [H_DIM, DPC], F32, kind="ExternalInput").ap(),
        nc.dram_tensor("wvT", [H_DIM, DPC], F32, kind="ExternalInput").ap(),
        nc.dram_tensor("woT", [DPC, H_DIM], F32, kind="ExternalInput").ap(),
        nc.dram_tensor("bqv", [DPC], F32, kind="ExternalInput").ap(),
        nc.dram_tensor("bkv", [DPC], F32, kind="ExternalInput").ap(),
        nc.dram_tensor("bvv", [DPC], F32, kind="ExternalInput").ap(),
        nc.dram_tensor("identity", [P, P], F32, kind="ExternalInput").ap(),
    ]
    outs = [nc.dram_tensor("outT", [H_DIM, TGT], F32, kind="ExternalOutput").ap()]
    with tile.TileContext(nc) as tc:
        _emit(tc, outs, ins)
    nc.compile()
    _prog_cache[key] = nc
    return nc


def _host_prep(query, key, value, attn_bias, attention_mask,
               Wq, bq, Wk, bk, Wv, bv, Wo, bo):
    """Build the 8 per-core input maps."""
    f = np.float32
    query = np.ascontiguousarray(np.asarray(query, f))
    key = np.asarray(key, f)
    value = np.asarray(value, f)
    attn_bias = np.asarray(attn_bias, f)
    mask = np.asarray(attention_mask)
    Wq = np.asarray(Wq, f); bq = np.asarray(bq, f)
    Wk = np.asarray(Wk, f); bk = np.asarray(bk, f)
    Wv = np.asarray(Wv, f); bv = np.asarray(bv, f)
    Wo = np.asarray(Wo, f)

    scale = f(1.0 / np.sqrt(HEAD_DIM))
    # fold mask into bias, transpose to [B, H, SRC, TGT]
    biasm = np.where(mask[:, None, :, :], f(NEG_MASK), attn_bias)
    biasmT = np.ascontiguousarray(biasm.transpose(0, 1, 3, 2))

    xqT = [np.ascontiguousarray(query[b].T) for b in range(B)]
    xkT = [np.ascontiguousarray(key[b].T) for b in range(B)]
    xvT = [np.ascontiguousarray(value[b].T) for b in range(B)]
    identity = np.eye(P, dtype=f)

    in_maps = []
    for c in range(N_CORES):
        b, g = divmod(c, N_CORES // B)
        hs = g * HPC
        he = hs + HPC
        ds_, de = hs * HEAD_DIM, he * HEAD_DIM
        in_maps.append({
            "xqT": xqT[b],
            "xkT": xkT[b],
            "xvT": xvT[b],
            "biasT": np.ascontiguousarray(biasmT[b, hs:he]),
            "wqT": np.ascontiguousarray((Wq[ds_:de] * scale).T),
            "wkT": np.ascontiguousarray(Wk[ds_:de].T),
            "wvT": np.ascontiguousarray(Wv[ds_:de].T),
            "woT": np.ascontiguousarray(Wo[:, ds_:de].T),
            "bqv": np.ascontiguousarray(bq[ds_:de] * scale),
            "bkv": np.ascontiguousarray(bk[ds_:de]),
            "bvv": np.ascontiguousarray(bv[ds_:de]),
            "identity": identity,
        })
    return in_maps


def _assemble(results, bo):
    bo = np.asarray(bo, np.float32)
    G = N_CORES // B
    out = np.empty((B, TGT, H_DIM), np.float32)
    for b in range(B):
        acc = results[b * G]["outT"].astype(np.float64)
        for g in range(1, G):
            acc = acc + results[b * G + g]["outT"]
        out[b] = acc.T.astype(np.float32) + bo
    return out


def kernel(**inputs):
    in_maps = _host_prep(**inputs)
    nc = _build_program()
    res = run_bass_kernel_spmd(nc, in_maps, core_ids=list(range(N_CORES)))
    return _assemble(res.results, inputs["bo"])

